# revision 7
# baseline (speedup 1.0000x reference)
"""SplineConv 2-layer GNN (nn_Net_23587960389976) on 8 trn2 NeuronCores.

Structure: 5 SPMD bass launches. All value arithmetic runs on device; the
host only shards, permutes by precomputed indices, and concatenates.

  L1: H = x_shard @ [W1_0|W1_1|root1|b1-row]  -> table1 shard (bf16) + root part
  L2: per-edge gather+basis-weight messages via one-hot matmuls (layer 1)
  L3: windowed segment-sum + mean + root + ELU + GEMM2 -> table2 shard + root2
  L4: gather+weight messages (layer 2)
  L5: segment-sum + mean + root2 + log_softmax

Per-core edge schedule is made SPMD-uniform with fixed capacities:
  gather: 5 tiles of 128 slots per 128-node src chunk (640 >= Poisson(512)+5.7s)
  segsum: 18 tiles of 128 slots per 64-dst window  (2304 >= Poisson(2048)+5.7s)
"""
import sys

sys.path.insert(0, "/opt/trn_rl_repo")

import numpy as np
import ml_dtypes

import concourse.bass as bass
import concourse.mybir as mybir

BF16 = ml_dtypes.bfloat16
F32 = np.float32

N_NODES = 50000
N_EDGES = 1600000
F_IN, F_HID, F_OUT = 1433, 16, 7
N_CORES = 8
NPC = N_NODES // N_CORES           # 6250
P = 128
N_CHUNKS = (N_NODES + P - 1) // P  # 391
NPAD = 392 * P                     # 50176 (chunk-padded)
KPAD = 1536                        # 1433+1 bias row, padded to 12*128
T0 = 5                             # gather tiles per chunk
N_G = N_CHUNKS * T0                # 1955 gather tiles
NG_PAD = ((N_G + 7) // 8) * 8      # 1960 (batch multiple)
N_CHUNKS_PAD = (NG_PAD + T0 - 1) // T0  # 392
WIN = 64
N_WIN = (NPC + WIN - 1) // WIN     # 98 windows
T1 = 18                            # segsum tiles per window
N_S = N_WIN * T1                   # 1764 segsum tiles
NT1 = 49                           # L1 node tiles (49*128 = 6272)
NPC_PAD = NT1 * P                  # 6272

# ------------------------------------------------------------------ patches
import concourse.tile as tile_mod
from concourse.tile import TileContext
from concourse.vector_clock import ScopedClock


def _patched_drain_and_barrier(self, tick_clock, wait_clock):
    nc = self.nc
    probe = nc.sync.nop(nofuse=True, hint="drain_wait_probe")
    wait_clock.add_sem_waits(probe.ins, ScopedClock({None: tick_clock.global_clock}))
    si = probe.ins.sync_info
    waits = list(si.on_wait) if si is not None else []
    if len(waits) > 1:
        probe.ins.sync_info = mybir.SyncInfo(on_update=list(si.on_update),
                                             on_wait=waits[:1])
        for w in waits[1:]:
            extra = nc.sync.nop(nofuse=True, hint="drain_wait_spill")
            extra.ins.sync_info = mybir.SyncInfo(on_update=[], on_wait=[w])
    nc.sync.drain()
    nc.all_engine_barrier()
    assert self.sems is not None
    popped = nc._tile_sem_poison_stack.pop()
    assert popped is self._sem_poison
    nc.clear_and_free_semaphores(list(self.sems.allocated().values()))
    nc.all_engine_barrier()


tile_mod.TileContext._drain_and_barrier = _patched_drain_and_barrier

_orig_lower = tile_mod.TileContext._lower_ordered_insts


def _split_multi_waits(ordered):
    for insts in ordered.values():
        out = []
        for inst in insts:
            si = getattr(inst, "sync_info", None)
            waits = list(si.on_wait) if si is not None and si.on_wait else []
            if len(waits) > 1:
                for k, w in enumerate(waits[:-1]):
                    out.append(mybir.InstNoOp(
                        name=f"{inst.name}-wsplit{k}", engine=inst.engine,
                        bass_nofuse=True,
                        sync_info=mybir.SyncInfo(on_wait=[w], on_update=[])))
                inst.sync_info = mybir.SyncInfo(on_wait=[waits[-1]],
                                                on_update=list(si.on_update))
            out.append(inst)
        insts[:] = out


def _patched_lower(self, ordered):
    _split_multi_waits(ordered)
    return _orig_lower(self, ordered)


tile_mod.TileContext._lower_ordered_insts = _patched_lower

# ------------------------------------------------------------------ launches

BATCH = 8


def build_L1():
    nc = bass.Bass()
    xT = nc.dram_tensor("xT", [KPAD, NPC_PAD], mybir.dt.bfloat16,
                        kind="ExternalInput")
    Wc = nc.dram_tensor("Wc", [KPAD, 48], mybir.dt.bfloat16,
                        kind="ExternalInput")
    table = nc.dram_tensor("table", [NPC_PAD, 32], mybir.dt.bfloat16,
                           kind="ExternalOutput")
    root = nc.dram_tensor("root", [NPC_PAD, 16], mybir.dt.float32,
                          kind="ExternalOutput")
    with TileContext(nc) as tc:
        with tc.tile_pool(name="w", bufs=1) as wpool, \
             tc.tile_pool(name="x", bufs=4) as xpool, \
             tc.tile_pool(name="o", bufs=3) as opool, \
             tc.tile_pool(name="ps", bufs=2, space="PSUM") as pspool:
            wt = wpool.tile([P, 12, 48], mybir.dt.bfloat16)
            nc.sync.dma_start(out=wt[:], in_=Wc[:].rearrange("(a p) f -> p a f", p=P))
            for t in range(NT1):
                ps = pspool.tile([P, 48], mybir.dt.float32, tag="ps")
                for k in range(12):
                    xt = xpool.tile([P, P], mybir.dt.bfloat16, tag="xt")
                    nc.sync.dma_start(
                        out=xt[:], in_=xT[k * P:(k + 1) * P, t * P:(t + 1) * P])
                    nc.tensor.matmul(out=ps[:], lhsT=xt[:], rhs=wt[:, k, :],
                                     start=(k == 0), stop=(k == 11))
                tb = opool.tile([P, 32], mybir.dt.bfloat16, tag="tb")
                nc.scalar.copy(out=tb[:], in_=ps[:, 0:32])
                nc.sync.dma_start(out=table[t * P:(t + 1) * P, :], in_=tb[:])
                rt = opool.tile([P, 16], mybir.dt.float32, tag="rt")
                nc.vector.tensor_copy(out=rt[:], in_=ps[:, 32:48])
                nc.sync.dma_start(out=root[t * P:(t + 1) * P, :], in_=rt[:])
    return nc


def build_gather(fdim, tab_cols, mcols):
    """L2 (fdim=16, tab_cols=32, mcols=16) / L4 (fdim=7, tab_cols=16, mcols=8)."""
    nc = bass.Bass()
    table = nc.dram_tensor("table", [NPAD, tab_cols], mybir.dt.bfloat16,
                           kind="ExternalInput")
    Et = nc.dram_tensor("Et", [NG_PAD * P, P], mybir.dt.bfloat16,
                        kind="ExternalInput")
    uin = nc.dram_tensor("u", [P, NG_PAD], mybir.dt.float32, kind="ExternalInput")
    msgs = nc.dram_tensor("msgs", [P, NG_PAD * mcols], mybir.dt.bfloat16,
                          kind="ExternalOutput")
    with TileContext(nc) as tc:
        with tc.tile_pool(name="tab", bufs=1) as tpool, \
             tc.tile_pool(name="et", bufs=3) as epool, \
             tc.tile_pool(name="u", bufs=1) as upool, \
             tc.tile_pool(name="m", bufs=3) as mpool, \
             tc.tile_pool(name="d", bufs=4) as dpool, \
             tc.tile_pool(name="ps", bufs=8, space="PSUM") as pspool:
            tab = tpool.tile([P, 392, tab_cols], mybir.dt.bfloat16)
            nc.sync.dma_start(out=tab[:],
                              in_=table[:].rearrange("(a p) f -> p a f", p=P))
            ut = upool.tile([P, NG_PAD], mybir.dt.float32)
            nc.sync.dma_start(out=ut[:], in_=uin[:])
            b0t = upool.tile([P, NG_PAD], mybir.dt.float32, tag="b0t")
            nc.vector.tensor_scalar(out=b0t[:], in0=ut[:], scalar1=-1.0,
                                    scalar2=1.0, op0=mybir.AluOpType.mult,
                                    op1=mybir.AluOpType.add)
            for t0 in range(0, NG_PAD, BATCH):
                et = epool.tile([P, BATCH, P], mybir.dt.bfloat16, tag="et")
                nc.sync.dma_start(
                    out=et[:],
                    in_=Et[t0 * P:(t0 + BATCH) * P, :].rearrange(
                        "(a p) c -> p a c", p=P))
                mt = mpool.tile([P, BATCH, mcols], mybir.dt.bfloat16, tag="mt")
                for j in range(BATCH):
                    t = t0 + j
                    ps = pspool.tile([P, 2 * fdim], mybir.dt.float32, tag="ps")
                    nc.tensor.matmul(out=ps[:], lhsT=et[:, j, :],
                                     rhs=tab[:, t // T0, 0:2 * fdim],
                                     start=True, stop=True)
                    d = dpool.tile([P, fdim], mybir.dt.float32, tag="d")
                    nc.vector.tensor_scalar(out=d[:], in0=ps[:, fdim:2 * fdim],
                                            scalar1=ut[:, t:t + 1], scalar2=None,
                                            op0=mybir.AluOpType.mult)
                    # msg = b0*g0 + u*g1
                    nc.vector.scalar_tensor_tensor(
                        out=mt[:, j, 0:fdim], in0=ps[:, 0:fdim],
                        scalar=b0t[:, t:t + 1], in1=d[:],
                        op0=mybir.AluOpType.mult, op1=mybir.AluOpType.add)
                nc.sync.dma_start(
                    out=msgs[:, t0 * mcols:(t0 + BATCH) * mcols],
                    in_=mt[:].rearrange("p a c -> p (a c)"))
    return nc


def build_segsum(fdim, mcols, layer1):
    nc = bass.Bass()
    msgs = nc.dram_tensor("msgs", [P, N_S * mcols], mybir.dt.bfloat16,
                          kind="ExternalInput")
    cin = nc.dram_tensor("c", [P, N_S], mybir.dt.float32, kind="ExternalInput")
    iotab = nc.dram_tensor("iotab", [P, BATCH * WIN], mybir.dt.float32,
                           kind="ExternalInput")
    invd = nc.dram_tensor("invd", [WIN, N_WIN], mybir.dt.float32,
                          kind="ExternalInput")
    root = nc.dram_tensor("root", [NPC_PAD, fdim], mybir.dt.float32,
                          kind="ExternalInput")
    if layer1:
        Wc2 = nc.dram_tensor("Wc2", [17, 21], mybir.dt.bfloat16,
                             kind="ExternalInput")
        id64 = nc.dram_tensor("id64", [WIN, WIN], mybir.dt.bfloat16,
                              kind="ExternalInput")
        table2 = nc.dram_tensor("table2", [NPC_PAD, 16], mybir.dt.bfloat16,
                                kind="ExternalOutput")
        root2 = nc.dram_tensor("root2", [NPC_PAD, 7], mybir.dt.float32,
                               kind="ExternalOutput")
    else:
        out = nc.dram_tensor("out", [NPC_PAD, 8], mybir.dt.float32,
                             kind="ExternalOutput")
    with TileContext(nc) as tc:
        with tc.tile_pool(name="m", bufs=3) as mpool, \
             tc.tile_pool(name="oh", bufs=3) as opool, \
             tc.tile_pool(name="agg", bufs=1) as apool, \
             tc.tile_pool(name="sc", bufs=1) as scpool, \
             tc.tile_pool(name="tmp", bufs=1) as tmppool, \
             tc.tile_pool(name="g2", bufs=4) as g2pool, \
             tc.tile_pool(name="ps", bufs=2, space="PSUM") as pspool, \
             tc.tile_pool(name="ps2", bufs=2, space="PSUM") as ps2pool:
            aggs = apool.tile([WIN, N_WIN, fdim], mybir.dt.float32)
            invt = scpool.tile([WIN, N_WIN], mybir.dt.float32, tag="invt")
            nc.sync.dma_start(out=invt[:], in_=invd[:])
            roott = scpool.tile([WIN, N_WIN, fdim], mybir.dt.float32, tag="roott")
            nc.sync.dma_start(
                out=roott[:],
                in_=root[0:N_WIN * WIN, :].rearrange("(a p) f -> p a f", p=WIN))
            ct = scpool.tile([P, N_S], mybir.dt.float32, tag="ct")
            nc.sync.dma_start(out=ct[:], in_=cin[:])
            iot = scpool.tile([P, BATCH, WIN], mybir.dt.float32, tag="iot")
            nc.sync.dma_start(out=iot[:],
                              in_=iotab[:].rearrange("p (a c) -> p a c", c=WIN))
            if layer1:
                w2t = scpool.tile([17, 21], mybir.dt.bfloat16, tag="w2t")
                nc.sync.dma_start(out=w2t[:], in_=Wc2[:])
                idt = scpool.tile([WIN, WIN], mybir.dt.bfloat16, tag="idt")
                nc.sync.dma_start(out=idt[:], in_=id64[:])
            # ---- streamed segment-sum
            cur_ps = None
            for t0 in range(0, N_S, BATCH):
                nb = min(BATCH, N_S - t0)
                mt = mpool.tile([P, BATCH, mcols], mybir.dt.bfloat16, tag="mt")
                nc.sync.dma_start(
                    out=mt[:, 0:nb, :],
                    in_=msgs[:, t0 * mcols:(t0 + nb) * mcols].rearrange(
                        "p (a c) -> p a c", c=mcols))
                oht = opool.tile([P, BATCH, WIN], mybir.dt.bfloat16, tag="oht")
                nc.vector.tensor_tensor(
                    out=oht[:, 0:nb, :], in0=iot[:, 0:nb, :],
                    in1=ct[:, t0:t0 + nb].to_broadcast([P, nb, WIN]),
                    op=mybir.AluOpType.is_equal)
                for j in range(nb):
                    t = t0 + j
                    w, tw = divmod(t, T1)
                    if tw == 0:
                        cur_ps = pspool.tile([WIN, fdim], mybir.dt.float32,
                                             tag="ps")
                    nc.tensor.matmul(out=cur_ps[:], lhsT=oht[:, j, :],
                                     rhs=mt[:, j, 0:fdim],
                                     start=(tw == 0), stop=(tw == T1 - 1))
                    if tw == T1 - 1:
                        nc.scalar.copy(out=aggs[:, w, :], in_=cur_ps[:])
            # ---- mean + root
            o1 = tmppool.tile([WIN, N_WIN, fdim], mybir.dt.float32, tag="o1")
            nc.vector.tensor_tensor(
                out=o1[:], in0=aggs[:],
                in1=invt[:].to_broadcast([WIN, N_WIN, fdim]),
                op=mybir.AluOpType.mult)
            nc.vector.tensor_add(out=o1[:], in0=o1[:], in1=roott[:])
            if layer1:
                # ELU
                mneg = tmppool.tile([WIN, N_WIN, fdim], mybir.dt.float32, tag="mn")
                nc.vector.tensor_scalar(out=mneg[:], in0=o1[:], scalar1=0.0,
                                        scalar2=None, op0=mybir.AluOpType.min)
                emt = tmppool.tile([WIN, N_WIN, fdim], mybir.dt.float32, tag="em")
                nc.scalar.activation(emt[:], mneg[:],
                                     mybir.ActivationFunctionType.Exp)
                rt = tmppool.tile([WIN, N_WIN, fdim], mybir.dt.float32, tag="rt")
                nc.vector.tensor_scalar(out=rt[:], in0=o1[:], scalar1=0.0,
                                        scalar2=None, op0=mybir.AluOpType.max)
                h1 = tmppool.tile([WIN, N_WIN, fdim], mybir.dt.bfloat16, tag="h1")
                nc.vector.scalar_tensor_tensor(
                    out=h1[:], in0=emt[:], scalar=-1.0, in1=rt[:],
                    op0=mybir.AluOpType.add, op1=mybir.AluOpType.add)
                # GEMM2 per window: h2 = [h1 | 1] @ Wc2
                t2 = g2pool.tile([WIN, N_WIN, 16], mybir.dt.bfloat16, tag="t2")
                r2 = g2pool.tile([WIN, N_WIN, 7], mybir.dt.float32, tag="r2")
                nc.vector.memset(t2[:], 0.0)
                for w in range(N_WIN):
                    psT = ps2pool.tile([16, WIN], mybir.dt.bfloat16, tag="psT")
                    nc.tensor.transpose(out=psT[:], in_=h1[:, w, :],
                                        identity=idt[:])
                    h1T = g2pool.tile([17, WIN], mybir.dt.bfloat16, tag="h1T")
                    nc.vector.memset(h1T[:], 1.0)
                    nc.scalar.copy(out=h1T[0:16, :], in_=psT[:])
                    ps2 = ps2pool.tile([WIN, 21], mybir.dt.float32, tag="ps2")
                    nc.tensor.matmul(out=ps2[:], lhsT=h1T[:], rhs=w2t[:],
                                     start=True, stop=True)
                    nc.scalar.copy(out=t2[:, w, 0:14], in_=ps2[:, 0:14])
                    nc.vector.tensor_copy(out=r2[:, w, :], in_=ps2[:, 14:21])
                nc.sync.dma_start(
                    out=table2[0:N_WIN * WIN, :].rearrange("(a p) f -> p a f", p=WIN),
                    in_=t2[:])
                nc.sync.dma_start(
                    out=root2[0:N_WIN * WIN, :].rearrange("(a p) f -> p a f", p=WIN),
                    in_=r2[:])
            else:
                # log_softmax over 7 logits
                mx = tmppool.tile([WIN, N_WIN], mybir.dt.float32, tag="mx")
                nc.vector.tensor_reduce(out=mx[:], in_=o1[:],
                                        axis=mybir.AxisListType.X,
                                        op=mybir.AluOpType.max)
                z = tmppool.tile([WIN, N_WIN, fdim], mybir.dt.float32, tag="z")
                nc.vector.tensor_sub(out=z[:], in0=o1[:],
                                     in1=mx[:].to_broadcast([WIN, N_WIN, fdim]))
                ez = tmppool.tile([WIN, N_WIN, fdim], mybir.dt.float32, tag="ez")
                nc.scalar.activation(ez[:], z[:],
                                     mybir.ActivationFunctionType.Exp)
                se = tmppool.tile([WIN, N_WIN], mybir.dt.float32, tag="se")
                nc.vector.tensor_reduce(out=se[:], in_=ez[:],
                                        axis=mybir.AxisListType.X,
                                        op=mybir.AluOpType.add)
                ls = tmppool.tile([WIN, N_WIN], mybir.dt.float32, tag="ls")
                nc.scalar.activation(ls[:], se[:],
                                     mybir.ActivationFunctionType.Ln)
                ot = tmppool.tile([WIN, N_WIN, 8], mybir.dt.float32, tag="ot")
                nc.vector.memset(ot[:], 0.0)
                nc.vector.tensor_sub(out=ot[:, :, 0:7], in0=z[:],
                                     in1=ls[:].to_broadcast([WIN, N_WIN, fdim]))
                nc.sync.dma_start(
                    out=out[0:N_WIN * WIN, :].rearrange("(a p) f -> p a f", p=WIN),
                    in_=ot[:])
    return nc


# ------------------------------------------------------------------ host prep


def _rank_within_group(group_sorted):
    """group_sorted: nondecreasing group ids; returns rank of each element
    within its group."""
    n = group_sorted.shape[0]
    if n == 0:
        return np.zeros(0, dtype=np.int64)
    first = np.searchsorted(group_sorted, group_sorted, side="left")
    return np.arange(n, dtype=np.int64) - first


def plan_core(src, dst_local, u):
    E = src.shape[0]
    # gather side (src-sorted, chunked)
    og = np.argsort(src, kind="stable")
    sg = src[og]
    chunk = sg // P
    rank = _rank_within_group(chunk)
    assert rank.max(initial=0) < T0 * P, "gather chunk overflow"
    slot = chunk * (T0 * P) + rank
    slot_of_edge = np.empty(E, dtype=np.int64)
    slot_of_edge[og] = slot
    tloc = slot // P
    col = slot % P
    nloc = sg - chunk * P
    Et = np.zeros((NG_PAD, P, P), dtype=BF16)
    Et[tloc, nloc, col] = BF16(1.0)
    u_slot = np.zeros((P, NG_PAD), dtype=F32)
    u_slot[col, tloc] = u[og]
    # segsum side (dst-sorted, windowed)
    os_ = np.argsort(dst_local, kind="stable")
    ds = dst_local[os_]
    win = ds // WIN
    rank_s = _rank_within_group(win)
    assert rank_s.max(initial=0) < T1 * P, "segsum window overflow"
    pos = win * (T1 * P) + rank_s
    tloc_s = pos // P
    row = pos % P
    cvals = np.full((P, N_S), -1.0, dtype=F32)
    cvals[row, tloc_s] = (ds - win * WIN).astype(F32)
    perm = np.zeros((P, N_S), dtype=np.int64)
    perm[row, tloc_s] = slot_of_edge[os_]
    deg = np.bincount(dst_local, minlength=NPC).astype(F32)
    inv = 1.0 / np.clip(deg, 1.0, None)
    inv_pad = np.zeros(N_WIN * WIN, dtype=F32)
    inv_pad[:NPC] = inv
    inv_wl = np.ascontiguousarray(inv_pad.reshape(N_WIN, WIN).T)
    return Et, u_slot, cvals, perm, inv_wl


# ------------------------------------------------------------------ driver


_NC_CACHE = {}


def _get_nc(name, builder):
    if name not in _NC_CACHE:
        _NC_CACHE[name] = builder()
    return _NC_CACHE[name]


def _run(name, builder, in_maps):
    from concourse.bass_utils import run_bass_kernel_spmd
    import time
    nc = _get_nc(name, builder)
    t0 = time.time()
    res = run_bass_kernel_spmd(nc, in_maps, list(range(N_CORES)))
    _run.times[name] = time.time() - t0
    return res.results


_run.times = {}


def kernel(x, edge_attr, edge_index, W1, root1, b1, W2, root2, b2):
    x = np.asarray(x, dtype=F32)
    u = np.asarray(edge_attr, dtype=F32).reshape(-1)
    ei = np.asarray(edge_index, dtype=np.int64)
    src_all, dst_all = ei[0], ei[1]

    # --- shard edges by dst owner core
    owner = dst_all // NPC
    plans = []
    for c in range(N_CORES):
        m = owner == c
        plans.append(plan_core(src_all[m], dst_all[m] - c * NPC, u[m]))

    # --- L1: GEMM
    Wc1 = np.zeros((KPAD, 48), dtype=BF16)
    Wc1[:F_IN, 0:16] = np.asarray(W1[0], dtype=BF16)
    Wc1[:F_IN, 16:32] = np.asarray(W1[1], dtype=BF16)
    Wc1[:F_IN, 32:48] = np.asarray(root1, dtype=BF16)
    Wc1[F_IN, 32:48] = np.asarray(b1, dtype=BF16)  # bias row
    xT = np.zeros((KPAD, N_NODES), dtype=BF16)
    xT[:F_IN, :] = np.asarray(x.T, dtype=BF16)
    xT[F_IN, :] = BF16(1.0)
    in1 = []
    for c in range(N_CORES):
        sh = np.zeros((KPAD, NPC_PAD), dtype=BF16)
        sh[:, :NPC] = xT[:, c * NPC:(c + 1) * NPC]
        in1.append({"xT": sh, "Wc": Wc1})
    import os
    dbg = bool(os.environ.get("KERNEL_DEBUG"))
    r1 = _run("L1", build_L1, in1)
    if dbg:
        Wcf = np.zeros((KPAD, 48), dtype=F32); Wcf[:] = Wc1.astype(F32)
        xTf = xT.astype(F32)
        H = xTf.T @ Wcf  # [N_NODES, 48]
        t1e = H[:NPC, 0:32]
        got = r1[0]["table"][:NPC].astype(F32)
        print("L1 table relerr:", np.abs(got - t1e).max() / (np.abs(t1e).max() + 1e-9))
        re = H[:NPC, 32:48]
        print("L1 root relerr:", np.abs(r1[0]["root"][:NPC] - re).max() / (np.abs(re).max() + 1e-9))
    table1 = np.zeros((NPAD, 32), dtype=BF16)
    roots = []
    for c in range(N_CORES):
        table1[c * NPC:(c + 1) * NPC] = r1[c]["table"][:NPC]
        roots.append(np.ascontiguousarray(r1[c]["root"]))

    # --- L2: gather layer 1
    in2 = [{"table": table1, "Et": plans[c][0].reshape(NG_PAD * P, P),
            "u": plans[c][1]} for c in range(N_CORES)]
    r2 = _run("L2", lambda: build_gather(16, 32, 16), in2)
    if dbg:
        c = 0
        tabf = table1.astype(F32)
        m = owner == c
        s0, d0, u0 = src_all[m], dst_all[m] - c * NPC, u[m]
        Etc, usl, cv, pm, ivw = plans[c]
        got = r2[c]["msgs"].reshape(P, NG_PAD, 16).astype(F32)
        # check a few real edges
        slot_map = {}
        og = np.argsort(s0, kind="stable"); sg = s0[og]
        ch = sg // P; rk = _rank_within_group(ch); sl = ch * (T0 * P) + rk
        exp_msg = (1 - u0[og])[:, None] * tabf[sg, 0:16] + u0[og][:, None] * tabf[sg, 16:32]
        gg = got[sl % P, sl // P]
        err = np.abs(gg - exp_msg).max() / (np.abs(exp_msg).max() + 1e-9)
        print("L2 msg relerr:", err)

    # --- L3: segsum + layer-1 tail
    Wc2 = np.zeros((17, 21), dtype=BF16)
    Wc2[:16, 0:7] = np.asarray(W2[0], dtype=BF16)
    Wc2[:16, 7:14] = np.asarray(W2[1], dtype=BF16)
    Wc2[:16, 14:21] = np.asarray(root2, dtype=BF16)
    Wc2[16, 14:21] = np.asarray(b2, dtype=BF16)
    iotab = np.tile(np.arange(WIN, dtype=F32)[None, :], (P, BATCH))
    id64 = np.eye(WIN, dtype=BF16)
    in3 = []
    for c in range(N_CORES):
        msgs = r2[c]["msgs"].reshape(P, NG_PAD, 16)
        flat = np.ascontiguousarray(msgs.transpose(1, 0, 2)).reshape(NG_PAD * P, 16)
        mp = flat[plans[c][3]]  # [P, N_S, 16]
        in3.append({"msgs": np.ascontiguousarray(mp).reshape(P, N_S * 16),
                    "c": plans[c][2], "iotab": iotab, "invd": plans[c][4],
                    "root": roots[c], "Wc2": Wc2, "id64": id64})
    r3 = _run("L3", lambda: build_segsum(16, 16, True), in3)
    table2 = np.zeros((NPAD, 16), dtype=BF16)
    roots2 = []
    for c in range(N_CORES):
        table2[c * NPC:(c + 1) * NPC] = r3[c]["table2"][:NPC]
        rr = np.zeros((NPC_PAD, 7), dtype=F32)
        rr[:] = r3[c]["root2"]
        roots2.append(rr)

    # --- L4: gather layer 2
    in4 = [{"table": table2, "Et": plans[c][0].reshape(NG_PAD * P, P),
            "u": plans[c][1]} for c in range(N_CORES)]
    r4 = _run("L4", lambda: build_gather(7, 16, 8), in4)

    # --- L5: segsum + final
    in5 = []
    for c in range(N_CORES):
        msgs = r4[c]["msgs"].reshape(P, NG_PAD, 8)
        flat = np.ascontiguousarray(msgs.transpose(1, 0, 2)).reshape(NG_PAD * P, 8)
        mp = flat[plans[c][3]]  # [P, N_S, 8]
        in5.append({"msgs": np.ascontiguousarray(mp).reshape(P, N_S * 8),
                    "c": plans[c][2], "iotab": iotab, "invd": plans[c][4],
                    "root": roots2[c]})
    r5 = _run("L5", lambda: build_segsum(7, 8, False), in5)

    out = np.zeros((N_NODES, F_OUT), dtype=F32)
    for c in range(N_CORES):
        out[c * NPC:(c + 1) * NPC] = r5[c]["out"][:NPC, :7]
    return out


# revision 12
# speedup vs baseline: 63807.8744x; 63807.8744x over previous
"""SplineConv 2-layer GNN (nn_Net_23587960389976) on 8 trn2 NeuronCores.

Structure: 5 SPMD bass launches. All value arithmetic runs on device; the
host only shards, permutes by precomputed indices, and concatenates.

  L1: H = x_shard @ [W1_0|W1_1|root1|b1-row]  -> table1 shard (bf16) + root part
  L2: per-edge gather+basis-weight messages via one-hot matmuls (layer 1)
  L3: windowed segment-sum + mean + root + ELU + GEMM2 -> table2 shard + root2
  L4: gather+weight messages (layer 2)
  L5: segment-sum + mean + root2 + log_softmax

Per-core edge schedule is made SPMD-uniform with fixed capacities:
  gather: 5 tiles of 128 slots per 128-node src chunk (640 >= Poisson(512)+5.7s)
  segsum: 18 tiles of 128 slots per 64-dst window  (2304 >= Poisson(2048)+5.7s)
"""
import sys

sys.path.insert(0, "/opt/trn_rl_repo")

import numpy as np
import ml_dtypes

import concourse.bass as bass
import concourse.mybir as mybir

BF16 = ml_dtypes.bfloat16
F32 = np.float32

N_NODES = 50000
N_EDGES = 1600000
F_IN, F_HID, F_OUT = 1433, 16, 7
N_CORES = 8
NPC = N_NODES // N_CORES           # 6250
P = 128
N_CHUNKS = (N_NODES + P - 1) // P  # 391
NPAD = 397 * P                     # 50816 (chunk-padded)
KPAD = 1536                        # 1433+1 bias row, padded to 12*128
T0 = 5                             # gather tiles per chunk
N_G = N_CHUNKS * T0                # 1955 gather tiles
NG_PAD = ((N_G + 31) // 32) * 32   # 1984 (batch multiple)
N_CHUNKS_PAD = (NG_PAD + T0 - 1) // T0  # 397
WIN = 64
N_WIN = (NPC + WIN - 1) // WIN     # 98 windows
T1 = 18                            # segsum tiles per window
N_S = N_WIN * T1                   # 1764 segsum tiles
NT1 = 49                           # L1 node tiles (49*128 = 6272)
NPC_PAD = NT1 * P                  # 6272

# ------------------------------------------------------------------ patches
import concourse.tile as tile_mod
from concourse.tile import TileContext
from concourse.vector_clock import ScopedClock


def _patched_drain_and_barrier(self, tick_clock, wait_clock):
    nc = self.nc
    probe = nc.sync.nop(nofuse=True, hint="drain_wait_probe")
    wait_clock.add_sem_waits(probe.ins, ScopedClock({None: tick_clock.global_clock}))
    si = probe.ins.sync_info
    waits = list(si.on_wait) if si is not None else []
    if len(waits) > 1:
        probe.ins.sync_info = mybir.SyncInfo(on_update=list(si.on_update),
                                             on_wait=waits[:1])
        for w in waits[1:]:
            extra = nc.sync.nop(nofuse=True, hint="drain_wait_spill")
            extra.ins.sync_info = mybir.SyncInfo(on_update=[], on_wait=[w])
    nc.sync.drain()
    nc.all_engine_barrier()
    assert self.sems is not None
    popped = nc._tile_sem_poison_stack.pop()
    assert popped is self._sem_poison
    nc.clear_and_free_semaphores(list(self.sems.allocated().values()))
    nc.all_engine_barrier()


tile_mod.TileContext._drain_and_barrier = _patched_drain_and_barrier

_orig_lower = tile_mod.TileContext._lower_ordered_insts


def _split_multi_waits(ordered):
    for insts in ordered.values():
        out = []
        for inst in insts:
            si = getattr(inst, "sync_info", None)
            waits = list(si.on_wait) if si is not None and si.on_wait else []
            if len(waits) > 1:
                for k, w in enumerate(waits[:-1]):
                    out.append(mybir.InstNoOp(
                        name=f"{inst.name}-wsplit{k}", engine=inst.engine,
                        bass_nofuse=True,
                        sync_info=mybir.SyncInfo(on_wait=[w], on_update=[])))
                inst.sync_info = mybir.SyncInfo(on_wait=[waits[-1]],
                                                on_update=list(si.on_update))
            out.append(inst)
        insts[:] = out


def _patched_lower(self, ordered):
    _split_multi_waits(ordered)
    return _orig_lower(self, ordered)


tile_mod.TileContext._lower_ordered_insts = _patched_lower

# ------------------------------------------------------------------ launches

BATCH = 32
SBATCH = 8


def build_L1():
    nc = bass.Bass()
    xT = nc.dram_tensor("xT", [KPAD, NPC_PAD], mybir.dt.bfloat16,
                        kind="ExternalInput")
    Wc = nc.dram_tensor("Wc", [KPAD, 48], mybir.dt.bfloat16,
                        kind="ExternalInput")
    table = nc.dram_tensor("table", [NPC_PAD, 32], mybir.dt.bfloat16,
                           kind="ExternalOutput")
    root = nc.dram_tensor("root", [NPC_PAD, 16], mybir.dt.float32,
                          kind="ExternalOutput")
    with TileContext(nc) as tc:
        with tc.tile_pool(name="w", bufs=1) as wpool, \
             tc.tile_pool(name="x", bufs=4) as xpool, \
             tc.tile_pool(name="o", bufs=3) as opool, \
             tc.tile_pool(name="ps", bufs=2, space="PSUM") as pspool:
            wt = wpool.tile([P, 12, 48], mybir.dt.bfloat16)
            nc.sync.dma_start(out=wt[:], in_=Wc[:].rearrange("(a p) f -> p a f", p=P))
            for t in range(NT1):
                ps = pspool.tile([P, 48], mybir.dt.float32, tag="ps")
                xt = xpool.tile([P, 12, P], mybir.dt.bfloat16, tag="xt")
                nc.sync.dma_start(
                    out=xt[:],
                    in_=xT[:, t * P:(t + 1) * P].rearrange("(a p) n -> p a n", p=P))
                for k in range(12):
                    nc.tensor.matmul(out=ps[:], lhsT=xt[:, k, :], rhs=wt[:, k, :],
                                     start=(k == 0), stop=(k == 11))
                tb = opool.tile([P, 32], mybir.dt.bfloat16, tag="tb")
                nc.scalar.copy(out=tb[:], in_=ps[:, 0:32])
                nc.sync.dma_start(out=table[t * P:(t + 1) * P, :], in_=tb[:])
                rt = opool.tile([P, 16], mybir.dt.float32, tag="rt")
                nc.vector.tensor_copy(out=rt[:], in_=ps[:, 32:48])
                nc.sync.dma_start(out=root[t * P:(t + 1) * P, :], in_=rt[:])
    return nc


def build_gather(fdim, tab_cols, mcols):
    """L2 (fdim=16, tab_cols=32, mcols=16) / L4 (fdim=7, tab_cols=16, mcols=8)."""
    nc = bass.Bass()
    table = nc.dram_tensor("table", [NPAD, tab_cols], mybir.dt.bfloat16,
                           kind="ExternalInput")
    Et = nc.dram_tensor("Et", [P, NG_PAD * P], mybir.dt.bfloat16,
                        kind="ExternalInput")
    uin = nc.dram_tensor("u", [P, NG_PAD], mybir.dt.float32, kind="ExternalInput")
    msgs = nc.dram_tensor("msgs", [P, NG_PAD * mcols], mybir.dt.bfloat16,
                          kind="ExternalOutput")
    with TileContext(nc) as tc:
        with tc.tile_pool(name="tab", bufs=1) as tpool, \
             tc.tile_pool(name="et", bufs=3) as epool, \
             tc.tile_pool(name="u", bufs=1) as upool, \
             tc.tile_pool(name="m", bufs=3) as mpool, \
             tc.tile_pool(name="d", bufs=4) as dpool, \
             tc.tile_pool(name="ps", bufs=8, space="PSUM") as pspool:
            tab = tpool.tile([P, 397, tab_cols], mybir.dt.bfloat16)
            nc.sync.dma_start(out=tab[:],
                              in_=table[:].rearrange("(a p) f -> p a f", p=P))
            ut = upool.tile([P, NG_PAD], mybir.dt.float32)
            nc.sync.dma_start(out=ut[:], in_=uin[:])
            b0t = upool.tile([P, NG_PAD], mybir.dt.float32, tag="b0t")
            nc.vector.tensor_scalar(out=b0t[:], in0=ut[:], scalar1=-1.0,
                                    scalar2=1.0, op0=mybir.AluOpType.mult,
                                    op1=mybir.AluOpType.add)
            for t0 in range(0, NG_PAD, BATCH):
                et = epool.tile([P, BATCH, P], mybir.dt.bfloat16, tag="et")
                nc.sync.dma_start(
                    out=et[:],
                    in_=Et[:, t0 * P:(t0 + BATCH) * P].rearrange(
                        "p (a c) -> p a c", c=P))
                mt = mpool.tile([P, BATCH, mcols], mybir.dt.bfloat16, tag="mt")
                for j in range(BATCH):
                    t = t0 + j
                    ps = pspool.tile([P, 2 * fdim], mybir.dt.float32, tag="ps")
                    nc.tensor.matmul(out=ps[:], lhsT=et[:, j, :],
                                     rhs=tab[:, t // T0, 0:2 * fdim],
                                     start=True, stop=True)
                    d = dpool.tile([P, fdim], mybir.dt.float32, tag="d")
                    nc.vector.tensor_scalar(out=d[:], in0=ps[:, fdim:2 * fdim],
                                            scalar1=ut[:, t:t + 1], scalar2=None,
                                            op0=mybir.AluOpType.mult)
                    # msg = b0*g0 + u*g1
                    nc.vector.scalar_tensor_tensor(
                        out=mt[:, j, 0:fdim], in0=ps[:, 0:fdim],
                        scalar=b0t[:, t:t + 1], in1=d[:],
                        op0=mybir.AluOpType.mult, op1=mybir.AluOpType.add)
                nc.scalar.dma_start(
                    out=msgs[:, t0 * mcols:(t0 + BATCH) * mcols],
                    in_=mt[:].rearrange("p a c -> p (a c)"))
    return nc


def build_segsum(fdim, mcols, layer1):
    nc = bass.Bass()
    msgs = nc.dram_tensor("msgs", [P, N_S * mcols], mybir.dt.bfloat16,
                          kind="ExternalInput")
    cin = nc.dram_tensor("c", [P, N_S], mybir.dt.float32, kind="ExternalInput")
    iotab = nc.dram_tensor("iotab", [P, SBATCH * WIN], mybir.dt.float32,
                           kind="ExternalInput")
    invd = nc.dram_tensor("invd", [WIN, N_WIN], mybir.dt.float32,
                          kind="ExternalInput")
    root = nc.dram_tensor("root", [NPC_PAD, fdim], mybir.dt.float32,
                          kind="ExternalInput")
    if layer1:
        Wc2 = nc.dram_tensor("Wc2", [17, 21], mybir.dt.bfloat16,
                             kind="ExternalInput")
        id64 = nc.dram_tensor("id64", [WIN, WIN], mybir.dt.bfloat16,
                              kind="ExternalInput")
        table2 = nc.dram_tensor("table2", [NPC_PAD, 16], mybir.dt.bfloat16,
                                kind="ExternalOutput")
        root2 = nc.dram_tensor("root2", [NPC_PAD, 7], mybir.dt.float32,
                               kind="ExternalOutput")
    else:
        out = nc.dram_tensor("out", [NPC_PAD, 8], mybir.dt.float32,
                             kind="ExternalOutput")
    with TileContext(nc) as tc:
        with tc.tile_pool(name="m", bufs=3) as mpool, \
             tc.tile_pool(name="oh", bufs=3) as opool, \
             tc.tile_pool(name="agg", bufs=1) as apool, \
             tc.tile_pool(name="sc", bufs=1) as scpool, \
             tc.tile_pool(name="tmp", bufs=1) as tmppool, \
             tc.tile_pool(name="g2", bufs=4) as g2pool, \
             tc.tile_pool(name="ps", bufs=2, space="PSUM") as pspool, \
             tc.tile_pool(name="ps2", bufs=2, space="PSUM") as ps2pool:
            aggs = apool.tile([WIN, N_WIN, fdim], mybir.dt.float32)
            invt = scpool.tile([WIN, N_WIN], mybir.dt.float32, tag="invt")
            nc.sync.dma_start(out=invt[:], in_=invd[:])
            roott = scpool.tile([WIN, N_WIN, fdim], mybir.dt.float32, tag="roott")
            nc.sync.dma_start(
                out=roott[:],
                in_=root[0:N_WIN * WIN, :].rearrange("(a p) f -> p a f", p=WIN))
            ct = scpool.tile([P, N_S], mybir.dt.float32, tag="ct")
            nc.sync.dma_start(out=ct[:], in_=cin[:])
            iot = scpool.tile([P, SBATCH, WIN], mybir.dt.float32, tag="iot")
            nc.sync.dma_start(out=iot[:],
                              in_=iotab[:].rearrange("p (a c) -> p a c", c=WIN))
            if layer1:
                w2t = scpool.tile([17, 21], mybir.dt.bfloat16, tag="w2t")
                nc.sync.dma_start(out=w2t[:], in_=Wc2[:])
                idt = scpool.tile([WIN, WIN], mybir.dt.bfloat16, tag="idt")
                nc.sync.dma_start(out=idt[:], in_=id64[:])
            # ---- streamed segment-sum
            cur_ps = None
            for t0 in range(0, N_S, SBATCH):
                nb = min(SBATCH, N_S - t0)
                mt = mpool.tile([P, SBATCH, mcols], mybir.dt.bfloat16, tag="mt")
                nc.sync.dma_start(
                    out=mt[:, 0:nb, :],
                    in_=msgs[:, t0 * mcols:(t0 + nb) * mcols].rearrange(
                        "p (a c) -> p a c", c=mcols))
                oht = opool.tile([P, SBATCH, WIN], mybir.dt.bfloat16, tag="oht")
                nc.vector.tensor_tensor(
                    out=oht[:, 0:nb, :], in0=iot[:, 0:nb, :],
                    in1=ct[:, t0:t0 + nb].to_broadcast([P, nb, WIN]),
                    op=mybir.AluOpType.is_equal)
                for j in range(nb):
                    t = t0 + j
                    w, tw = divmod(t, T1)
                    if tw == 0:
                        cur_ps = pspool.tile([WIN, fdim], mybir.dt.float32,
                                             tag="ps")
                    nc.tensor.matmul(out=cur_ps[:], lhsT=oht[:, j, :],
                                     rhs=mt[:, j, 0:fdim],
                                     start=(tw == 0), stop=(tw == T1 - 1))
                    if tw == T1 - 1:
                        nc.scalar.copy(out=aggs[:, w, :], in_=cur_ps[:])
            # ---- mean + root
            o1 = tmppool.tile([WIN, N_WIN, fdim], mybir.dt.float32, tag="o1")
            nc.vector.tensor_tensor(
                out=o1[:], in0=aggs[:],
                in1=invt[:].to_broadcast([WIN, N_WIN, fdim]),
                op=mybir.AluOpType.mult)
            nc.vector.tensor_add(out=o1[:], in0=o1[:], in1=roott[:])
            if layer1:
                # ELU
                mneg = tmppool.tile([WIN, N_WIN, fdim], mybir.dt.float32, tag="mn")
                nc.vector.tensor_scalar(out=mneg[:], in0=o1[:], scalar1=0.0,
                                        scalar2=None, op0=mybir.AluOpType.min)
                emt = tmppool.tile([WIN, N_WIN, fdim], mybir.dt.float32, tag="em")
                nc.scalar.activation(emt[:], mneg[:],
                                     mybir.ActivationFunctionType.Exp)
                rt = tmppool.tile([WIN, N_WIN, fdim], mybir.dt.float32, tag="rt")
                nc.vector.tensor_scalar(out=rt[:], in0=o1[:], scalar1=0.0,
                                        scalar2=None, op0=mybir.AluOpType.max)
                h1 = tmppool.tile([WIN, N_WIN, fdim], mybir.dt.bfloat16, tag="h1")
                nc.vector.scalar_tensor_tensor(
                    out=h1[:], in0=emt[:], scalar=-1.0, in1=rt[:],
                    op0=mybir.AluOpType.add, op1=mybir.AluOpType.add)
                # GEMM2 per window: h2 = [h1 | 1] @ Wc2
                t2 = g2pool.tile([WIN, N_WIN, 16], mybir.dt.bfloat16, tag="t2")
                r2 = g2pool.tile([WIN, N_WIN, 7], mybir.dt.float32, tag="r2")
                nc.vector.memset(t2[:], 0.0)
                for w in range(N_WIN):
                    psT = ps2pool.tile([16, WIN], mybir.dt.bfloat16, tag="psT")
                    nc.tensor.transpose(out=psT[:], in_=h1[:, w, :],
                                        identity=idt[:])
                    h1T = g2pool.tile([17, WIN], mybir.dt.bfloat16, tag="h1T")
                    nc.vector.memset(h1T[:], 1.0)
                    nc.scalar.copy(out=h1T[0:16, :], in_=psT[:])
                    ps2 = ps2pool.tile([WIN, 21], mybir.dt.float32, tag="ps2")
                    nc.tensor.matmul(out=ps2[:], lhsT=h1T[:], rhs=w2t[:],
                                     start=True, stop=True)
                    nc.scalar.copy(out=t2[:, w, 0:14], in_=ps2[:, 0:14])
                    nc.vector.tensor_copy(out=r2[:, w, :], in_=ps2[:, 14:21])
                nc.sync.dma_start(
                    out=table2[0:N_WIN * WIN, :].rearrange("(a p) f -> p a f", p=WIN),
                    in_=t2[:])
                nc.sync.dma_start(
                    out=root2[0:N_WIN * WIN, :].rearrange("(a p) f -> p a f", p=WIN),
                    in_=r2[:])
            else:
                # log_softmax over 7 logits
                mx = tmppool.tile([WIN, N_WIN], mybir.dt.float32, tag="mx")
                nc.vector.tensor_reduce(out=mx[:], in_=o1[:],
                                        axis=mybir.AxisListType.X,
                                        op=mybir.AluOpType.max)
                z = tmppool.tile([WIN, N_WIN, fdim], mybir.dt.float32, tag="z")
                nc.vector.tensor_sub(out=z[:], in0=o1[:],
                                     in1=mx[:].to_broadcast([WIN, N_WIN, fdim]))
                ez = tmppool.tile([WIN, N_WIN, fdim], mybir.dt.float32, tag="ez")
                nc.scalar.activation(ez[:], z[:],
                                     mybir.ActivationFunctionType.Exp)
                se = tmppool.tile([WIN, N_WIN], mybir.dt.float32, tag="se")
                nc.vector.tensor_reduce(out=se[:], in_=ez[:],
                                        axis=mybir.AxisListType.X,
                                        op=mybir.AluOpType.add)
                ls = tmppool.tile([WIN, N_WIN], mybir.dt.float32, tag="ls")
                nc.scalar.activation(ls[:], se[:],
                                     mybir.ActivationFunctionType.Ln)
                ot = tmppool.tile([WIN, N_WIN, 8], mybir.dt.float32, tag="ot")
                nc.vector.memset(ot[:], 0.0)
                nc.vector.tensor_sub(out=ot[:, :, 0:7], in0=z[:],
                                     in1=ls[:].to_broadcast([WIN, N_WIN, fdim]))
                nc.sync.dma_start(
                    out=out[0:N_WIN * WIN, :].rearrange("(a p) f -> p a f", p=WIN),
                    in_=ot[:])
    return nc


# ------------------------------------------------------------------ host prep


def _rank_within_group(group_sorted):
    """group_sorted: nondecreasing group ids; returns rank of each element
    within its group."""
    n = group_sorted.shape[0]
    if n == 0:
        return np.zeros(0, dtype=np.int64)
    first = np.searchsorted(group_sorted, group_sorted, side="left")
    return np.arange(n, dtype=np.int64) - first


def plan_core(src, dst_local, u):
    E = src.shape[0]
    # gather side (src-sorted, chunked)
    og = np.argsort(src, kind="stable")
    sg = src[og]
    chunk = sg // P
    rank = _rank_within_group(chunk)
    assert rank.max(initial=0) < T0 * P, "gather chunk overflow"
    slot = chunk * (T0 * P) + rank
    slot_of_edge = np.empty(E, dtype=np.int64)
    slot_of_edge[og] = slot
    tloc = slot // P
    col = slot % P
    nloc = sg - chunk * P
    Et = np.zeros((NG_PAD, P, P), dtype=BF16)
    Et[tloc, nloc, col] = BF16(1.0)
    u_slot = np.zeros((P, NG_PAD), dtype=F32)
    u_slot[col, tloc] = u[og]
    # segsum side (dst-sorted, windowed)
    os_ = np.argsort(dst_local, kind="stable")
    ds = dst_local[os_]
    win = ds // WIN
    rank_s = _rank_within_group(win)
    assert rank_s.max(initial=0) < T1 * P, "segsum window overflow"
    pos = win * (T1 * P) + rank_s
    tloc_s = pos // P
    row = pos % P
    cvals = np.full((P, N_S), -1.0, dtype=F32)
    cvals[row, tloc_s] = (ds - win * WIN).astype(F32)
    perm = np.zeros((P, N_S), dtype=np.int64)
    perm[row, tloc_s] = slot_of_edge[os_]
    deg = np.bincount(dst_local, minlength=NPC).astype(F32)
    inv = 1.0 / np.clip(deg, 1.0, None)
    inv_pad = np.zeros(N_WIN * WIN, dtype=F32)
    inv_pad[:NPC] = inv
    inv_wl = np.ascontiguousarray(inv_pad.reshape(N_WIN, WIN).T)
    return Et, u_slot, cvals, perm, inv_wl


def _et_pmaj(Et):
    return np.ascontiguousarray(Et.transpose(1, 0, 2)).reshape(P, NG_PAD * P)


# ------------------------------------------------------------------ driver


_NC_CACHE = {}


def _get_nc(name, builder):
    if name not in _NC_CACHE:
        _NC_CACHE[name] = builder()
    return _NC_CACHE[name]


def _run(name, builder, in_maps):
    from concourse.bass_utils import run_bass_kernel_spmd
    import time
    nc = _get_nc(name, builder)
    t0 = time.time()
    res = run_bass_kernel_spmd(nc, in_maps, list(range(N_CORES)))
    _run.times[name] = time.time() - t0
    return res.results


_run.times = {}


def kernel(x, edge_attr, edge_index, W1, root1, b1, W2, root2, b2):
    x = np.asarray(x, dtype=F32)
    u = np.asarray(edge_attr, dtype=F32).reshape(-1)
    ei = np.asarray(edge_index, dtype=np.int64)
    src_all, dst_all = ei[0], ei[1]

    # --- shard edges by dst owner core
    owner = dst_all // NPC
    plans = []
    for c in range(N_CORES):
        m = owner == c
        plans.append(plan_core(src_all[m], dst_all[m] - c * NPC, u[m]))

    # --- L1: GEMM
    Wc1 = np.zeros((KPAD, 48), dtype=BF16)
    Wc1[:F_IN, 0:16] = np.asarray(W1[0], dtype=BF16)
    Wc1[:F_IN, 16:32] = np.asarray(W1[1], dtype=BF16)
    Wc1[:F_IN, 32:48] = np.asarray(root1, dtype=BF16)
    Wc1[F_IN, 32:48] = np.asarray(b1, dtype=BF16)  # bias row
    xT = np.zeros((KPAD, N_NODES), dtype=BF16)
    xT[:F_IN, :] = np.asarray(x.T, dtype=BF16)
    xT[F_IN, :] = BF16(1.0)
    in1 = []
    for c in range(N_CORES):
        sh = np.zeros((KPAD, NPC_PAD), dtype=BF16)
        sh[:, :NPC] = xT[:, c * NPC:(c + 1) * NPC]
        in1.append({"xT": sh, "Wc": Wc1})
    import os
    dbg = bool(os.environ.get("KERNEL_DEBUG"))
    r1 = _run("L1", build_L1, in1)
    if dbg:
        Wcf = np.zeros((KPAD, 48), dtype=F32); Wcf[:] = Wc1.astype(F32)
        xTf = xT.astype(F32)
        H = xTf.T @ Wcf  # [N_NODES, 48]
        t1e = H[:NPC, 0:32]
        got = r1[0]["table"][:NPC].astype(F32)
        print("L1 table relerr:", np.abs(got - t1e).max() / (np.abs(t1e).max() + 1e-9))
        re = H[:NPC, 32:48]
        print("L1 root relerr:", np.abs(r1[0]["root"][:NPC] - re).max() / (np.abs(re).max() + 1e-9))
    table1 = np.zeros((NPAD, 32), dtype=BF16)
    roots = []
    for c in range(N_CORES):
        table1[c * NPC:(c + 1) * NPC] = r1[c]["table"][:NPC]
        roots.append(np.ascontiguousarray(r1[c]["root"]))

    # --- L2: gather layer 1
    in2 = [{"table": table1, "Et": _et_pmaj(plans[c][0]),
            "u": plans[c][1]} for c in range(N_CORES)]
    r2 = _run("L2", lambda: build_gather(16, 32, 16), in2)
    if dbg:
        c = 0
        tabf = table1.astype(F32)
        m = owner == c
        s0, d0, u0 = src_all[m], dst_all[m] - c * NPC, u[m]
        Etc, usl, cv, pm, ivw = plans[c]
        got = r2[c]["msgs"].reshape(P, NG_PAD, 16).astype(F32)
        # check a few real edges
        slot_map = {}
        og = np.argsort(s0, kind="stable"); sg = s0[og]
        ch = sg // P; rk = _rank_within_group(ch); sl = ch * (T0 * P) + rk
        exp_msg = (1 - u0[og])[:, None] * tabf[sg, 0:16] + u0[og][:, None] * tabf[sg, 16:32]
        gg = got[sl % P, sl // P]
        err = np.abs(gg - exp_msg).max() / (np.abs(exp_msg).max() + 1e-9)
        print("L2 msg relerr:", err)

    # --- L3: segsum + layer-1 tail
    Wc2 = np.zeros((17, 21), dtype=BF16)
    Wc2[:16, 0:7] = np.asarray(W2[0], dtype=BF16)
    Wc2[:16, 7:14] = np.asarray(W2[1], dtype=BF16)
    Wc2[:16, 14:21] = np.asarray(root2, dtype=BF16)
    Wc2[16, 14:21] = np.asarray(b2, dtype=BF16)
    iotab = np.tile(np.arange(WIN, dtype=F32)[None, :], (P, SBATCH))
    id64 = np.eye(WIN, dtype=BF16)
    in3 = []
    for c in range(N_CORES):
        msgs = r2[c]["msgs"].reshape(P, NG_PAD, 16)
        flat = np.ascontiguousarray(msgs.transpose(1, 0, 2)).reshape(NG_PAD * P, 16)
        mp = flat[plans[c][3]]  # [P, N_S, 16]
        in3.append({"msgs": np.ascontiguousarray(mp).reshape(P, N_S * 16),
                    "c": plans[c][2], "iotab": iotab, "invd": plans[c][4],
                    "root": roots[c], "Wc2": Wc2, "id64": id64})
    r3 = _run("L3", lambda: build_segsum(16, 16, True), in3)
    table2 = np.zeros((NPAD, 16), dtype=BF16)
    roots2 = []
    for c in range(N_CORES):
        table2[c * NPC:(c + 1) * NPC] = r3[c]["table2"][:NPC]
        rr = np.zeros((NPC_PAD, 7), dtype=F32)
        rr[:] = r3[c]["root2"]
        roots2.append(rr)

    # --- L4: gather layer 2
    in4 = [{"table": table2, "Et": _et_pmaj(plans[c][0]),
            "u": plans[c][1]} for c in range(N_CORES)]
    r4 = _run("L4", lambda: build_gather(7, 16, 8), in4)

    # --- L5: segsum + final
    in5 = []
    for c in range(N_CORES):
        msgs = r4[c]["msgs"].reshape(P, NG_PAD, 8)
        flat = np.ascontiguousarray(msgs.transpose(1, 0, 2)).reshape(NG_PAD * P, 8)
        mp = flat[plans[c][3]]  # [P, N_S, 8]
        in5.append({"msgs": np.ascontiguousarray(mp).reshape(P, N_S * 8),
                    "c": plans[c][2], "iotab": iotab, "invd": plans[c][4],
                    "root": roots2[c]})
    r5 = _run("L5", lambda: build_segsum(7, 8, False), in5)

    out = np.zeros((N_NODES, F_OUT), dtype=F32)
    for c in range(N_CORES):
        out[c * NPC:(c + 1) * NPC] = r5[c]["out"][:NPC, :7]
    return out


# revision 13
# speedup vs baseline: 75211.3466x; 1.1787x over previous
"""SplineConv 2-layer GNN (nn_Net_23587960389976) on 8 trn2 NeuronCores.

Structure: 5 SPMD bass launches. All value arithmetic runs on device; the
host only shards, permutes by precomputed indices, and concatenates.

  L1: H = x_shard @ [W1_0|W1_1|root1|b1-row]  -> table1 shard (bf16) + root part
  L2: per-edge gather+basis-weight messages via one-hot matmuls (layer 1)
  L3: windowed segment-sum + mean + root + ELU + GEMM2 -> table2 shard + root2
  L4: gather+weight messages (layer 2)
  L5: segment-sum + mean + root2 + log_softmax

Per-core edge schedule is made SPMD-uniform with fixed capacities:
  gather: 5 tiles of 128 slots per 128-node src chunk (640 >= Poisson(512)+5.7s)
  segsum: 18 tiles of 128 slots per 64-dst window  (2304 >= Poisson(2048)+5.7s)
"""
import sys

sys.path.insert(0, "/opt/trn_rl_repo")

import numpy as np
import ml_dtypes

import concourse.bass as bass
import concourse.mybir as mybir

BF16 = ml_dtypes.bfloat16
F32 = np.float32

N_NODES = 50000
N_EDGES = 1600000
F_IN, F_HID, F_OUT = 1433, 16, 7
N_CORES = 8
NPC = N_NODES // N_CORES           # 6250
P = 128
N_CHUNKS = (N_NODES + P - 1) // P  # 391
NPAD = 397 * P                     # 50816 (chunk-padded)
KPAD = 1536                        # 1433+1 bias row, padded to 12*128
T0 = 5                             # gather tiles per chunk
N_G = N_CHUNKS * T0                # 1955 gather tiles
NG_PAD = ((N_G + 31) // 32) * 32   # 1984 (batch multiple)
N_CHUNKS_PAD = (NG_PAD + T0 - 1) // T0  # 397
WIN = 64
N_WIN = (NPC + WIN - 1) // WIN     # 98 windows
T1 = 18                            # segsum tiles per window
N_S = N_WIN * T1                   # 1764 segsum tiles
NT1 = 49                           # L1 node tiles (49*128 = 6272)
NPC_PAD = NT1 * P                  # 6272

# ------------------------------------------------------------------ patches
import concourse.tile as tile_mod
from concourse.tile import TileContext
from concourse.vector_clock import ScopedClock


def _patched_drain_and_barrier(self, tick_clock, wait_clock):
    nc = self.nc
    probe = nc.sync.nop(nofuse=True, hint="drain_wait_probe")
    wait_clock.add_sem_waits(probe.ins, ScopedClock({None: tick_clock.global_clock}))
    si = probe.ins.sync_info
    waits = list(si.on_wait) if si is not None else []
    if len(waits) > 1:
        probe.ins.sync_info = mybir.SyncInfo(on_update=list(si.on_update),
                                             on_wait=waits[:1])
        for w in waits[1:]:
            extra = nc.sync.nop(nofuse=True, hint="drain_wait_spill")
            extra.ins.sync_info = mybir.SyncInfo(on_update=[], on_wait=[w])
    nc.sync.drain()
    nc.all_engine_barrier()
    assert self.sems is not None
    popped = nc._tile_sem_poison_stack.pop()
    assert popped is self._sem_poison
    nc.clear_and_free_semaphores(list(self.sems.allocated().values()))
    nc.all_engine_barrier()


tile_mod.TileContext._drain_and_barrier = _patched_drain_and_barrier

_orig_lower = tile_mod.TileContext._lower_ordered_insts


def _split_multi_waits(ordered):
    for insts in ordered.values():
        out = []
        for inst in insts:
            si = getattr(inst, "sync_info", None)
            waits = list(si.on_wait) if si is not None and si.on_wait else []
            if len(waits) > 1:
                for k, w in enumerate(waits[:-1]):
                    out.append(mybir.InstNoOp(
                        name=f"{inst.name}-wsplit{k}", engine=inst.engine,
                        bass_nofuse=True,
                        sync_info=mybir.SyncInfo(on_wait=[w], on_update=[])))
                inst.sync_info = mybir.SyncInfo(on_wait=[waits[-1]],
                                                on_update=list(si.on_update))
            out.append(inst)
        insts[:] = out


def _patched_lower(self, ordered):
    _split_multi_waits(ordered)
    return _orig_lower(self, ordered)


tile_mod.TileContext._lower_ordered_insts = _patched_lower

# ------------------------------------------------------------------ launches

BATCH = 32
SBATCH = 8


def build_L1():
    nc = bass.Bass()
    xT = nc.dram_tensor("xT", [KPAD, NPC_PAD], mybir.dt.bfloat16,
                        kind="ExternalInput")
    Wc = nc.dram_tensor("Wc", [KPAD, 48], mybir.dt.bfloat16,
                        kind="ExternalInput")
    table = nc.dram_tensor("table", [NPC_PAD, 32], mybir.dt.bfloat16,
                           kind="ExternalOutput")
    root = nc.dram_tensor("root", [NPC_PAD, 16], mybir.dt.float32,
                          kind="ExternalOutput")
    with TileContext(nc) as tc:
        with tc.tile_pool(name="w", bufs=1) as wpool, \
             tc.tile_pool(name="x", bufs=4) as xpool, \
             tc.tile_pool(name="o", bufs=3) as opool, \
             tc.tile_pool(name="ps", bufs=2, space="PSUM") as pspool:
            wt = wpool.tile([P, 12, 48], mybir.dt.bfloat16)
            nc.sync.dma_start(out=wt[:], in_=Wc[:].rearrange("(a p) f -> p a f", p=P))
            for t in range(NT1):
                ps = pspool.tile([P, 48], mybir.dt.float32, tag="ps")
                xt = xpool.tile([P, 12, P], mybir.dt.bfloat16, tag="xt")
                nc.sync.dma_start(
                    out=xt[:],
                    in_=xT[:, t * P:(t + 1) * P].rearrange("(a p) n -> p a n", p=P))
                for k in range(12):
                    nc.tensor.matmul(out=ps[:], lhsT=xt[:, k, :], rhs=wt[:, k, :],
                                     start=(k == 0), stop=(k == 11))
                tb = opool.tile([P, 32], mybir.dt.bfloat16, tag="tb")
                nc.scalar.copy(out=tb[:], in_=ps[:, 0:32])
                nc.sync.dma_start(out=table[t * P:(t + 1) * P, :], in_=tb[:])
                rt = opool.tile([P, 16], mybir.dt.float32, tag="rt")
                nc.vector.tensor_copy(out=rt[:], in_=ps[:, 32:48])
                nc.sync.dma_start(out=root[t * P:(t + 1) * P, :], in_=rt[:])
    return nc


def build_gather(fdim, tab_cols, mcols):
    """L2 (fdim=16, tab_cols=32, mcols=16) / L4 (fdim=7, tab_cols=16, mcols=8)."""
    nc = bass.Bass()
    table = nc.dram_tensor("table", [NPAD, tab_cols], mybir.dt.bfloat16,
                           kind="ExternalInput")
    Et = nc.dram_tensor("Et", [P, NG_PAD * P], mybir.dt.bfloat16,
                        kind="ExternalInput")
    uin = nc.dram_tensor("u", [P, NG_PAD], mybir.dt.float32, kind="ExternalInput")
    msgs = nc.dram_tensor("msgs", [P, NG_PAD * mcols], mybir.dt.bfloat16,
                          kind="ExternalOutput")
    with TileContext(nc) as tc:
        with tc.tile_pool(name="tab", bufs=1) as tpool, \
             tc.tile_pool(name="et", bufs=3) as epool, \
             tc.tile_pool(name="u", bufs=1) as upool, \
             tc.tile_pool(name="m", bufs=3) as mpool, \
             tc.tile_pool(name="d", bufs=4) as dpool, \
             tc.tile_pool(name="ps", bufs=8, space="PSUM") as pspool:
            tab = tpool.tile([P, 397, tab_cols], mybir.dt.bfloat16)
            nc.sync.dma_start(out=tab[:],
                              in_=table[:].rearrange("(a p) f -> p a f", p=P))
            ut = upool.tile([P, NG_PAD], mybir.dt.float32)
            nc.sync.dma_start(out=ut[:], in_=uin[:])
            b0t = upool.tile([P, NG_PAD], mybir.dt.float32, tag="b0t")
            nc.vector.tensor_scalar(out=b0t[:], in0=ut[:], scalar1=-1.0,
                                    scalar2=1.0, op0=mybir.AluOpType.mult,
                                    op1=mybir.AluOpType.add)
            for t0 in range(0, NG_PAD, BATCH):
                et = epool.tile([P, BATCH, P], mybir.dt.bfloat16, tag="et")
                nc.sync.dma_start(
                    out=et[:],
                    in_=Et[:, t0 * P:(t0 + BATCH) * P].rearrange(
                        "p (a c) -> p a c", c=P))
                mt = mpool.tile([P, BATCH, mcols], mybir.dt.bfloat16, tag="mt")
                for j in range(BATCH):
                    t = t0 + j
                    ps = pspool.tile([P, 2 * fdim], mybir.dt.float32, tag="ps")
                    nc.tensor.matmul(out=ps[:], lhsT=et[:, j, :],
                                     rhs=tab[:, t // T0, 0:2 * fdim],
                                     start=True, stop=True)
                    d = dpool.tile([P, fdim], mybir.dt.float32, tag="d")
                    nc.scalar.activation(d[:], ps[:, fdim:2 * fdim],
                                         mybir.ActivationFunctionType.Copy,
                                         scale=ut[:, t:t + 1])
                    # msg = b0*g0 + u*g1
                    nc.vector.scalar_tensor_tensor(
                        out=mt[:, j, 0:fdim], in0=ps[:, 0:fdim],
                        scalar=b0t[:, t:t + 1], in1=d[:],
                        op0=mybir.AluOpType.mult, op1=mybir.AluOpType.add)
                nc.scalar.dma_start(
                    out=msgs[:, t0 * mcols:(t0 + BATCH) * mcols],
                    in_=mt[:].rearrange("p a c -> p (a c)"))
    return nc


def build_segsum(fdim, mcols, layer1):
    nc = bass.Bass()
    msgs = nc.dram_tensor("msgs", [P, N_S * mcols], mybir.dt.bfloat16,
                          kind="ExternalInput")
    cin = nc.dram_tensor("c", [P, N_S], mybir.dt.float32, kind="ExternalInput")
    iotab = nc.dram_tensor("iotab", [P, SBATCH * WIN], mybir.dt.float32,
                           kind="ExternalInput")
    invd = nc.dram_tensor("invd", [WIN, N_WIN], mybir.dt.float32,
                          kind="ExternalInput")
    root = nc.dram_tensor("root", [NPC_PAD, fdim], mybir.dt.float32,
                          kind="ExternalInput")
    if layer1:
        Wc2 = nc.dram_tensor("Wc2", [17, 21], mybir.dt.bfloat16,
                             kind="ExternalInput")
        id64 = nc.dram_tensor("id64", [WIN, WIN], mybir.dt.bfloat16,
                              kind="ExternalInput")
        table2 = nc.dram_tensor("table2", [NPC_PAD, 16], mybir.dt.bfloat16,
                                kind="ExternalOutput")
        root2 = nc.dram_tensor("root2", [NPC_PAD, 7], mybir.dt.float32,
                               kind="ExternalOutput")
    else:
        out = nc.dram_tensor("out", [NPC_PAD, 8], mybir.dt.float32,
                             kind="ExternalOutput")
    with TileContext(nc) as tc:
        with tc.tile_pool(name="m", bufs=3) as mpool, \
             tc.tile_pool(name="oh", bufs=3) as opool, \
             tc.tile_pool(name="agg", bufs=1) as apool, \
             tc.tile_pool(name="sc", bufs=1) as scpool, \
             tc.tile_pool(name="tmp", bufs=1) as tmppool, \
             tc.tile_pool(name="g2", bufs=4) as g2pool, \
             tc.tile_pool(name="ps", bufs=2, space="PSUM") as pspool, \
             tc.tile_pool(name="ps2", bufs=2, space="PSUM") as ps2pool:
            aggs = apool.tile([WIN, N_WIN, fdim], mybir.dt.float32)
            invt = scpool.tile([WIN, N_WIN], mybir.dt.float32, tag="invt")
            nc.sync.dma_start(out=invt[:], in_=invd[:])
            roott = scpool.tile([WIN, N_WIN, fdim], mybir.dt.float32, tag="roott")
            nc.sync.dma_start(
                out=roott[:],
                in_=root[0:N_WIN * WIN, :].rearrange("(a p) f -> p a f", p=WIN))
            ct = scpool.tile([P, N_S], mybir.dt.float32, tag="ct")
            nc.sync.dma_start(out=ct[:], in_=cin[:])
            iot = scpool.tile([P, SBATCH, WIN], mybir.dt.float32, tag="iot")
            nc.sync.dma_start(out=iot[:],
                              in_=iotab[:].rearrange("p (a c) -> p a c", c=WIN))
            if layer1:
                w2t = scpool.tile([17, 21], mybir.dt.bfloat16, tag="w2t")
                nc.sync.dma_start(out=w2t[:], in_=Wc2[:])
                idt = scpool.tile([WIN, WIN], mybir.dt.bfloat16, tag="idt")
                nc.sync.dma_start(out=idt[:], in_=id64[:])
            # ---- streamed segment-sum
            cur_ps = None
            for t0 in range(0, N_S, SBATCH):
                nb = min(SBATCH, N_S - t0)
                mt = mpool.tile([P, SBATCH, mcols], mybir.dt.bfloat16, tag="mt")
                nc.sync.dma_start(
                    out=mt[:, 0:nb, :],
                    in_=msgs[:, t0 * mcols:(t0 + nb) * mcols].rearrange(
                        "p (a c) -> p a c", c=mcols))
                oht = opool.tile([P, SBATCH, WIN], mybir.dt.bfloat16, tag="oht")
                nc.vector.tensor_tensor(
                    out=oht[:, 0:nb, :], in0=iot[:, 0:nb, :],
                    in1=ct[:, t0:t0 + nb].to_broadcast([P, nb, WIN]),
                    op=mybir.AluOpType.is_equal)
                for j in range(nb):
                    t = t0 + j
                    w, tw = divmod(t, T1)
                    if tw == 0:
                        cur_ps = pspool.tile([WIN, fdim], mybir.dt.float32,
                                             tag="ps")
                    nc.tensor.matmul(out=cur_ps[:], lhsT=oht[:, j, :],
                                     rhs=mt[:, j, 0:fdim],
                                     start=(tw == 0), stop=(tw == T1 - 1))
                    if tw == T1 - 1:
                        nc.scalar.copy(out=aggs[:, w, :], in_=cur_ps[:])
            # ---- mean + root
            o1 = tmppool.tile([WIN, N_WIN, fdim], mybir.dt.float32, tag="o1")
            nc.vector.tensor_tensor(
                out=o1[:], in0=aggs[:],
                in1=invt[:].to_broadcast([WIN, N_WIN, fdim]),
                op=mybir.AluOpType.mult)
            nc.vector.tensor_add(out=o1[:], in0=o1[:], in1=roott[:])
            if layer1:
                # ELU
                mneg = tmppool.tile([WIN, N_WIN, fdim], mybir.dt.float32, tag="mn")
                nc.vector.tensor_scalar(out=mneg[:], in0=o1[:], scalar1=0.0,
                                        scalar2=None, op0=mybir.AluOpType.min)
                emt = tmppool.tile([WIN, N_WIN, fdim], mybir.dt.float32, tag="em")
                nc.scalar.activation(emt[:], mneg[:],
                                     mybir.ActivationFunctionType.Exp)
                rt = tmppool.tile([WIN, N_WIN, fdim], mybir.dt.float32, tag="rt")
                nc.vector.tensor_scalar(out=rt[:], in0=o1[:], scalar1=0.0,
                                        scalar2=None, op0=mybir.AluOpType.max)
                h1 = tmppool.tile([WIN, N_WIN, fdim], mybir.dt.bfloat16, tag="h1")
                nc.vector.scalar_tensor_tensor(
                    out=h1[:], in0=emt[:], scalar=-1.0, in1=rt[:],
                    op0=mybir.AluOpType.add, op1=mybir.AluOpType.add)
                # GEMM2 per window: h2 = [h1 | 1] @ Wc2
                t2 = g2pool.tile([WIN, N_WIN, 16], mybir.dt.bfloat16, tag="t2")
                r2 = g2pool.tile([WIN, N_WIN, 7], mybir.dt.float32, tag="r2")
                nc.vector.memset(t2[:], 0.0)
                for w in range(N_WIN):
                    psT = ps2pool.tile([16, WIN], mybir.dt.bfloat16, tag="psT")
                    nc.tensor.transpose(out=psT[:], in_=h1[:, w, :],
                                        identity=idt[:])
                    h1T = g2pool.tile([17, WIN], mybir.dt.bfloat16, tag="h1T")
                    nc.vector.memset(h1T[:], 1.0)
                    nc.scalar.copy(out=h1T[0:16, :], in_=psT[:])
                    ps2 = ps2pool.tile([WIN, 21], mybir.dt.float32, tag="ps2")
                    nc.tensor.matmul(out=ps2[:], lhsT=h1T[:], rhs=w2t[:],
                                     start=True, stop=True)
                    nc.scalar.copy(out=t2[:, w, 0:14], in_=ps2[:, 0:14])
                    nc.vector.tensor_copy(out=r2[:, w, :], in_=ps2[:, 14:21])
                nc.sync.dma_start(
                    out=table2[0:N_WIN * WIN, :].rearrange("(a p) f -> p a f", p=WIN),
                    in_=t2[:])
                nc.sync.dma_start(
                    out=root2[0:N_WIN * WIN, :].rearrange("(a p) f -> p a f", p=WIN),
                    in_=r2[:])
            else:
                # log_softmax over 7 logits
                mx = tmppool.tile([WIN, N_WIN], mybir.dt.float32, tag="mx")
                nc.vector.tensor_reduce(out=mx[:], in_=o1[:],
                                        axis=mybir.AxisListType.X,
                                        op=mybir.AluOpType.max)
                z = tmppool.tile([WIN, N_WIN, fdim], mybir.dt.float32, tag="z")
                nc.vector.tensor_sub(out=z[:], in0=o1[:],
                                     in1=mx[:].to_broadcast([WIN, N_WIN, fdim]))
                ez = tmppool.tile([WIN, N_WIN, fdim], mybir.dt.float32, tag="ez")
                nc.scalar.activation(ez[:], z[:],
                                     mybir.ActivationFunctionType.Exp)
                se = tmppool.tile([WIN, N_WIN], mybir.dt.float32, tag="se")
                nc.vector.tensor_reduce(out=se[:], in_=ez[:],
                                        axis=mybir.AxisListType.X,
                                        op=mybir.AluOpType.add)
                ls = tmppool.tile([WIN, N_WIN], mybir.dt.float32, tag="ls")
                nc.scalar.activation(ls[:], se[:],
                                     mybir.ActivationFunctionType.Ln)
                ot = tmppool.tile([WIN, N_WIN, 8], mybir.dt.float32, tag="ot")
                nc.vector.memset(ot[:], 0.0)
                nc.vector.tensor_sub(out=ot[:, :, 0:7], in0=z[:],
                                     in1=ls[:].to_broadcast([WIN, N_WIN, fdim]))
                nc.sync.dma_start(
                    out=out[0:N_WIN * WIN, :].rearrange("(a p) f -> p a f", p=WIN),
                    in_=ot[:])
    return nc


# ------------------------------------------------------------------ host prep


def _rank_within_group(group_sorted):
    """group_sorted: nondecreasing group ids; returns rank of each element
    within its group."""
    n = group_sorted.shape[0]
    if n == 0:
        return np.zeros(0, dtype=np.int64)
    first = np.searchsorted(group_sorted, group_sorted, side="left")
    return np.arange(n, dtype=np.int64) - first


def plan_core(src, dst_local, u):
    E = src.shape[0]
    # gather side (src-sorted, chunked)
    og = np.argsort(src, kind="stable")
    sg = src[og]
    chunk = sg // P
    rank = _rank_within_group(chunk)
    assert rank.max(initial=0) < T0 * P, "gather chunk overflow"
    slot = chunk * (T0 * P) + rank
    slot_of_edge = np.empty(E, dtype=np.int64)
    slot_of_edge[og] = slot
    tloc = slot // P
    col = slot % P
    nloc = sg - chunk * P
    Et = np.zeros((NG_PAD, P, P), dtype=BF16)
    Et[tloc, nloc, col] = BF16(1.0)
    u_slot = np.zeros((P, NG_PAD), dtype=F32)
    u_slot[col, tloc] = u[og]
    # segsum side (dst-sorted, windowed)
    os_ = np.argsort(dst_local, kind="stable")
    ds = dst_local[os_]
    win = ds // WIN
    rank_s = _rank_within_group(win)
    assert rank_s.max(initial=0) < T1 * P, "segsum window overflow"
    pos = win * (T1 * P) + rank_s
    tloc_s = pos // P
    row = pos % P
    cvals = np.full((P, N_S), -1.0, dtype=F32)
    cvals[row, tloc_s] = (ds - win * WIN).astype(F32)
    perm = np.zeros((P, N_S), dtype=np.int64)
    perm[row, tloc_s] = slot_of_edge[os_]
    deg = np.bincount(dst_local, minlength=NPC).astype(F32)
    inv = 1.0 / np.clip(deg, 1.0, None)
    inv_pad = np.zeros(N_WIN * WIN, dtype=F32)
    inv_pad[:NPC] = inv
    inv_wl = np.ascontiguousarray(inv_pad.reshape(N_WIN, WIN).T)
    return Et, u_slot, cvals, perm, inv_wl


def _et_pmaj(Et):
    return np.ascontiguousarray(Et.transpose(1, 0, 2)).reshape(P, NG_PAD * P)


# ------------------------------------------------------------------ driver


_NC_CACHE = {}


def _get_nc(name, builder):
    if name not in _NC_CACHE:
        _NC_CACHE[name] = builder()
    return _NC_CACHE[name]


def _run(name, builder, in_maps):
    from concourse.bass_utils import run_bass_kernel_spmd
    import time
    nc = _get_nc(name, builder)
    t0 = time.time()
    res = run_bass_kernel_spmd(nc, in_maps, list(range(N_CORES)))
    _run.times[name] = time.time() - t0
    return res.results


_run.times = {}


def kernel(x, edge_attr, edge_index, W1, root1, b1, W2, root2, b2):
    x = np.asarray(x, dtype=F32)
    u = np.asarray(edge_attr, dtype=F32).reshape(-1)
    ei = np.asarray(edge_index, dtype=np.int64)
    src_all, dst_all = ei[0], ei[1]

    # --- shard edges by dst owner core
    owner = dst_all // NPC
    plans = []
    for c in range(N_CORES):
        m = owner == c
        plans.append(plan_core(src_all[m], dst_all[m] - c * NPC, u[m]))

    # --- L1: GEMM
    Wc1 = np.zeros((KPAD, 48), dtype=BF16)
    Wc1[:F_IN, 0:16] = np.asarray(W1[0], dtype=BF16)
    Wc1[:F_IN, 16:32] = np.asarray(W1[1], dtype=BF16)
    Wc1[:F_IN, 32:48] = np.asarray(root1, dtype=BF16)
    Wc1[F_IN, 32:48] = np.asarray(b1, dtype=BF16)  # bias row
    xT = np.zeros((KPAD, N_NODES), dtype=BF16)
    xT[:F_IN, :] = np.asarray(x.T, dtype=BF16)
    xT[F_IN, :] = BF16(1.0)
    in1 = []
    for c in range(N_CORES):
        sh = np.zeros((KPAD, NPC_PAD), dtype=BF16)
        sh[:, :NPC] = xT[:, c * NPC:(c + 1) * NPC]
        in1.append({"xT": sh, "Wc": Wc1})
    import os
    dbg = bool(os.environ.get("KERNEL_DEBUG"))
    r1 = _run("L1", build_L1, in1)
    if dbg:
        Wcf = np.zeros((KPAD, 48), dtype=F32); Wcf[:] = Wc1.astype(F32)
        xTf = xT.astype(F32)
        H = xTf.T @ Wcf  # [N_NODES, 48]
        t1e = H[:NPC, 0:32]
        got = r1[0]["table"][:NPC].astype(F32)
        print("L1 table relerr:", np.abs(got - t1e).max() / (np.abs(t1e).max() + 1e-9))
        re = H[:NPC, 32:48]
        print("L1 root relerr:", np.abs(r1[0]["root"][:NPC] - re).max() / (np.abs(re).max() + 1e-9))
    table1 = np.zeros((NPAD, 32), dtype=BF16)
    roots = []
    for c in range(N_CORES):
        table1[c * NPC:(c + 1) * NPC] = r1[c]["table"][:NPC]
        roots.append(np.ascontiguousarray(r1[c]["root"]))

    # --- L2: gather layer 1
    in2 = [{"table": table1, "Et": _et_pmaj(plans[c][0]),
            "u": plans[c][1]} for c in range(N_CORES)]
    r2 = _run("L2", lambda: build_gather(16, 32, 16), in2)
    if dbg:
        c = 0
        tabf = table1.astype(F32)
        m = owner == c
        s0, d0, u0 = src_all[m], dst_all[m] - c * NPC, u[m]
        Etc, usl, cv, pm, ivw = plans[c]
        got = r2[c]["msgs"].reshape(P, NG_PAD, 16).astype(F32)
        # check a few real edges
        slot_map = {}
        og = np.argsort(s0, kind="stable"); sg = s0[og]
        ch = sg // P; rk = _rank_within_group(ch); sl = ch * (T0 * P) + rk
        exp_msg = (1 - u0[og])[:, None] * tabf[sg, 0:16] + u0[og][:, None] * tabf[sg, 16:32]
        gg = got[sl % P, sl // P]
        err = np.abs(gg - exp_msg).max() / (np.abs(exp_msg).max() + 1e-9)
        print("L2 msg relerr:", err)

    # --- L3: segsum + layer-1 tail
    Wc2 = np.zeros((17, 21), dtype=BF16)
    Wc2[:16, 0:7] = np.asarray(W2[0], dtype=BF16)
    Wc2[:16, 7:14] = np.asarray(W2[1], dtype=BF16)
    Wc2[:16, 14:21] = np.asarray(root2, dtype=BF16)
    Wc2[16, 14:21] = np.asarray(b2, dtype=BF16)
    iotab = np.tile(np.arange(WIN, dtype=F32)[None, :], (P, SBATCH))
    id64 = np.eye(WIN, dtype=BF16)
    in3 = []
    for c in range(N_CORES):
        msgs = r2[c]["msgs"].reshape(P, NG_PAD, 16)
        flat = np.ascontiguousarray(msgs.transpose(1, 0, 2)).reshape(NG_PAD * P, 16)
        mp = flat[plans[c][3]]  # [P, N_S, 16]
        in3.append({"msgs": np.ascontiguousarray(mp).reshape(P, N_S * 16),
                    "c": plans[c][2], "iotab": iotab, "invd": plans[c][4],
                    "root": roots[c], "Wc2": Wc2, "id64": id64})
    r3 = _run("L3", lambda: build_segsum(16, 16, True), in3)
    table2 = np.zeros((NPAD, 16), dtype=BF16)
    roots2 = []
    for c in range(N_CORES):
        table2[c * NPC:(c + 1) * NPC] = r3[c]["table2"][:NPC]
        rr = np.zeros((NPC_PAD, 7), dtype=F32)
        rr[:] = r3[c]["root2"]
        roots2.append(rr)

    # --- L4: gather layer 2
    in4 = [{"table": table2, "Et": _et_pmaj(plans[c][0]),
            "u": plans[c][1]} for c in range(N_CORES)]
    r4 = _run("L4", lambda: build_gather(7, 16, 8), in4)

    # --- L5: segsum + final
    in5 = []
    for c in range(N_CORES):
        msgs = r4[c]["msgs"].reshape(P, NG_PAD, 8)
        flat = np.ascontiguousarray(msgs.transpose(1, 0, 2)).reshape(NG_PAD * P, 8)
        mp = flat[plans[c][3]]  # [P, N_S, 8]
        in5.append({"msgs": np.ascontiguousarray(mp).reshape(P, N_S * 8),
                    "c": plans[c][2], "iotab": iotab, "invd": plans[c][4],
                    "root": roots2[c]})
    r5 = _run("L5", lambda: build_segsum(7, 8, False), in5)

    out = np.zeros((N_NODES, F_OUT), dtype=F32)
    for c in range(N_CORES):
        out[c * NPC:(c + 1) * NPC] = r5[c]["out"][:NPC, :7]
    return out
